# revision 14
# baseline (speedup 1.0000x reference)
"""BitConv1d Trainium2 kernel.

Computes, for x:(8,512,8192) f32, weight:(512,512,7) f32, gamma:(512,) f32:
  rms  = sqrt(mean(x^2, channel) + 1e-6)          (per b,t)
  xn   = x / rms * gamma
  s    = max(|xn|) over the FULL batch  (clamped to >= 1e-5)
  q    = round(clip(xn/s*127, -128, 127))         (8-bit act quant, STE forward)
  ws   = max(mean(|w|), 1e-5); wq = round(clip(w/ws, -1, 1))  (ternary weights)
  out  = conv1d(q * s/127, wq, pad 3) * ws

Strategy: data-parallel over batch across 8 NeuronCores (1 batch element per
core), weights replicated. The activation-quant global max uses an on-device
AllReduce(max) of max(xn^2). The conv runs as 28 shifted bf16 matmuls per
output tile with exact integer arithmetic (q in [-127,127] and wq in {-1,0,1}
are exact in bf16; f32 PSUM accumulation of integers < 2^24 is exact), so the
conv result equals the integer conv scaled by s*ws/127. Rounding uses the
(x + 1.5*2^23) - 1.5*2^23 trick (round-half-even, matching jnp.round).
"""

import sys

sys.path.insert(0, "/opt/trn_rl_repo")

import numpy as np

N_CORES = 8
B, C, T = 8, 512, 8192
CO, K = 512, 7
CI_CHUNKS = 4  # 512 in-channels / 128 partitions
CB_BLOCKS = 4  # 512 out-channels / 128 partitions
TT = 512  # time-tile (columns per matmul)
PAD = 3  # conv padding

EPS_NORM = 1e-6
EPS_SCALE = 1e-5
QP = 127.0
C_MAGIC = 12582912.0  # 1.5 * 2^23 : (x + C) - C == round-half-even(x)
W_COUNT = CO * C * K

_CACHE = {}


def _build(n_cores: int, t_len: int):
    import contextlib

    import concourse.bacc as bacc
    import concourse.bass as bass
    import concourse.tile as tile
    from concourse import bass_isa, mybir

    f32 = mybir.dt.float32
    bf16 = mybir.dt.bfloat16
    Alu = mybir.AluOpType
    Act = mybir.ActivationFunctionType
    ts = bass.ts

    NT = t_len // TT  # time tiles
    WQ_F = CB_BLOCKS * K * CI_CHUNKS * 128  # 14336
    NW = 16  # weight streaming chunks
    WCH = WQ_F // NW  # 896 columns per chunk

    nc = bacc.Bacc("TRN2", target_bir_lowering=False, debug=False,
                   num_devices=n_cores)

    x_t = nc.dram_tensor("x", [C, t_len], f32, kind="ExternalInput")
    wt_t = nc.dram_tensor("wt", [128, WQ_F], f32, kind="ExternalInput")
    g_t = nc.dram_tensor("g", [C], f32, kind="ExternalInput")
    out_t = nc.dram_tensor("out", [CO, t_len], f32, kind="ExternalOutput")

    xv = x_t[:].rearrange("(c p) t -> p c t", p=128)  # chunk-major channels

    with tile.TileContext(nc) as tc:
        with contextlib.ExitStack() as stk:
            singles = stk.enter_context(tc.tile_pool(name="singles", bufs=1))
            scr = stk.enter_context(tc.tile_pool(name="scr", bufs=3))
            bncp = stk.enter_context(tc.tile_pool(name="bncp", bufs=2))
            rmathp = stk.enter_context(tc.tile_pool(name="rmathp", bufs=5))
            scp = stk.enter_context(tc.tile_pool(name="scp", bufs=14))
            amaxp = stk.enter_context(tc.tile_pool(name="amaxp", bufs=2))
            rowp = stk.enter_context(tc.tile_pool(name="rowp", bufs=1))
            wstga = stk.enter_context(tc.tile_pool(name="wstga", bufs=2))
            dramp = stk.enter_context(
                tc.tile_pool(name="dram", bufs=1, space="DRAM"))
            ps_small = stk.enter_context(
                tc.tile_pool(name="ps_small", bufs=2, space="PSUM"))
            ps_mb = stk.enter_context(
                tc.tile_pool(name="ps_mb", bufs=2, space="PSUM"))
            ps_conv = stk.enter_context(
                tc.tile_pool(name="ps_conv", bufs=4, space="PSUM"))

            ones_col = singles.tile([128, 1], f32)
            nc.vector.memset(ones_col[:], 1.0)
            eps_col = singles.tile([128, 1], f32)
            nc.vector.memset(eps_col[:], EPS_NORM)
            zero_col = singles.tile([128, 1], f32)
            nc.vector.memset(zero_col[:], 0.0)
            g_row = singles.tile([1, C], f32)
            nc.sync.dma_start(g_row[:], g_t[:].rearrange("(a d) -> a d", a=1))

            cc_in = dramp.tile([128], f32)
            cc_out = dramp.tile([128], f32)

            FW = t_len // 128  # per-t arrays reshaped to (128, FW)
            PPT = TT // FW  # partitions covered by one t-tile

            # ---------------- phase 1a: sum of squares per t ---------------
            # x stays resident in SBUF through phase 1b.
            with tc.tile_pool(name="xres", bufs=1) as xres:
                x_sb = xres.tile([128, CI_CHUNKS, t_len], f32)
                rcol = singles.tile([128, FW], f32)  # channel-summed x^2
                for j in range(NT):
                    nc.sync.dma_start(x_sb[:, :, ts(j, TT)], xv[:, :, ts(j, TT)])
                    ssq = ps_small.tile([1, TT], f32, tag="ssq")
                    for ci in range(CI_CHUNKS):
                        x2 = scr.tile([128, TT], f32, tag="scr")
                        nc.scalar.activation(x2[:], x_sb[:, ci, ts(j, TT)],
                                             Act.Square)
                        nc.tensor.matmul(ssq[:], ones_col[:], x2[:],
                                         start=(ci == 0),
                                         stop=(ci == CI_CHUNKS - 1))
                    sbounce = bncp.tile([1, TT], f32, tag="sbounce")
                    nc.scalar.copy(sbounce[:], ssq[:])
                    nc.sync.dma_start(rcol[PPT * j:PPT * (j + 1), :],
                                      sbounce[:])

                # ---- r = 1/(2*rms), one Newton refinement of sqrt ----
                mcol = rmathp.tile([128, FW], f32, tag="rmath")
                s0 = rmathp.tile([128, FW], f32, tag="rmath")
                tdiv = rmathp.tile([128, FW], f32, tag="rmath")
                rhalf = rmathp.tile([128, FW], f32, tag="rmath")
                nc.vector.tensor_scalar(mcol[:], rcol[:], 1.0 / C, EPS_NORM,
                                        op0=Alu.mult, op1=Alu.add)
                nc.scalar.activation(s0[:], rcol[:], Act.Sqrt,
                                     bias=eps_col[:], scale=1.0 / C)
                nc.vector.reciprocal(tdiv[:], s0[:])
                nc.vector.tensor_tensor(tdiv[:], mcol[:], tdiv[:], op=Alu.mult)
                nc.vector.tensor_tensor(tdiv[:], tdiv[:], s0[:], op=Alu.add)
                nc.vector.reciprocal(rhalf[:], tdiv[:])  # 1/(2*rms)

                g2_row = singles.tile([1, C], f32)
                nc.vector.tensor_scalar_mul(g2_row[:], g_row[:], 2.0)
                # squared variants for the max pass (max of xn^2)
                g2sq_row = singles.tile([1, C], f32)
                nc.scalar.activation(g2sq_row[:], g2_row[:], Act.Square)
                rsqc = rmathp.tile([128, FW], f32, tag="rmath2")
                nc.scalar.activation(rsqc[:], rhalf[:], Act.Square)
                rsq_row = rowp.tile([1, t_len], f32, tag="trow")
                nc.sync.dma_start(rsq_row[0:1, :], rsqc[:])

                # ---- weight pass A: sum(|w|) for mean (overlaps 1b) ----
                wsacc = None
                for e in range(NW):
                    wt_e = wstga.tile([128, WCH], f32, tag="wstga")
                    nc.sync.dma_start(wt_e[:], wt_t[:, ts(e, WCH)])
                    wsq = scp.tile([128, 1], f32, tag="sc")
                    nc.scalar.activation(wt_e[:], wt_e[:], Act.Abs,
                                         accum_out=wsq[:])
                    if wsacc is None:
                        wsacc = wsq
                    else:
                        nxt = scp.tile([128, 1], f32, tag="sc")
                        nc.vector.tensor_tensor(nxt[:], wsacc[:], wsq[:],
                                                op=Alu.add)
                        wsacc = nxt
                wsum_ps = ps_small.tile([1, 1], f32, tag="ssq")
                nc.tensor.matmul(wsum_ps[:], wsacc[:], ones_col[:, 0:1],
                                 start=True, stop=True)
                wscale = scp.tile([1, 1], f32, tag="sc")
                nc.scalar.copy(wscale[:], wsum_ps[:])
                nc.vector.tensor_scalar(wscale[:], wscale[:], 1.0 / W_COUNT,
                                        EPS_SCALE, op0=Alu.mult, op1=Alu.max)
                winv = scp.tile([1, 1], f32, tag="sc")
                nc.vector.reciprocal(winv[:], wscale[:])
                winv_col = scp.tile([128, 1], f32, tag="sc")
                nc.gpsimd.partition_broadcast(winv_col[:], winv[:])

                # ---------------- phase 1b: local max of xn^2 --------------
                coll = singles.tile([128, NT * CI_CHUNKS], f32)
                for j in range(NT):
                    for ci in range(CI_CHUNKS):
                        mbsq = ps_mb.tile([128, TT], f32, tag="mb")
                        nc.tensor.matmul(mbsq[:], g2sq_row[0:1, ts(ci, 128)],
                                         rsq_row[0:1, ts(j, TT)],
                                         start=True, stop=True)
                        x2b = scr.tile([128, TT], f32, tag="scrb")
                        nc.scalar.activation(x2b[:], x_sb[:, ci, ts(j, TT)],
                                             Act.Square)
                        u = scr.tile([128, TT], f32, tag="scrb")
                        nc.vector.tensor_tensor(u[:], x2b[:], mbsq[:],
                                                op=Alu.mult)
                        idx = j * CI_CHUNKS + ci
                        nc.vector.tensor_reduce(
                            coll[:, idx:idx + 1], u[:],
                            axis=mybir.AxisListType.X, op=Alu.max)
                prev = amaxp.tile([128, 1], f32, tag="amax")
                nc.vector.tensor_reduce(prev[:], coll[:],
                                        axis=mybir.AxisListType.X, op=Alu.max)

            # x_sb freed here.
            amax_all = scp.tile([128, 1], f32, tag="sc")
            nc.gpsimd.partition_all_reduce(amax_all[:], prev[:], channels=128,
                                           reduce_op=bass_isa.ReduceOp.max)
            nc.sync.dma_start(cc_in[:], amax_all[:])
            if n_cores > 1:
                nc.gpsimd.collective_compute(
                    "AllReduce", Alu.max,
                    replica_groups=[list(range(n_cores))],
                    ins=[cc_in[:].opt()], outs=[cc_out[:].opt()])
            else:
                nc.sync.dma_start(cc_out[:], cc_in[:])

            v_raw = scp.tile([1, 1], f32, tag="sc")
            nc.sync.dma_start(v_raw[0:1, 0:1],
                              cc_out[0:1].rearrange("(a d) -> a d", a=1))
            # scale = max(sqrt(v), 1e-5), sqrt via spline + one Newton step
            nc.vector.tensor_scalar_max(v_raw[:], v_raw[:], 1e-12)
            s0a = scp.tile([1, 1], f32, tag="sc")
            nc.scalar.activation(s0a[:], v_raw[:], Act.Sqrt,
                                 bias=zero_col[0:1, :], scale=1.0)
            tda = scp.tile([1, 1], f32, tag="sc")
            nc.vector.reciprocal(tda[:], s0a[:])
            nc.vector.tensor_tensor(tda[:], v_raw[:], tda[:], op=Alu.mult)
            nc.vector.tensor_tensor(tda[:], tda[:], s0a[:], op=Alu.add)
            qscale = scp.tile([1, 1], f32, tag="sc")
            nc.vector.tensor_scalar(qscale[:], tda[:], 0.5, EPS_SCALE,
                                    op0=Alu.mult, op1=Alu.max)
            qinv = scp.tile([1, 1], f32, tag="sc")
            nc.vector.reciprocal(qinv[:], qscale[:])
            q254 = scp.tile([1, 1], f32, tag="sc")
            nc.vector.tensor_scalar_mul(q254[:], qinv[:], 2.0 * QP)
            g2q_row = singles.tile([1, C], f32)
            nc.vector.tensor_scalar_mul(g2q_row[:], g_row[:], q254[:])
            # final output scale = wscale * qscale / 127
            fs = scp.tile([1, 1], f32, tag="sc")
            nc.vector.tensor_tensor(fs[:], wscale[:], qscale[:], op=Alu.mult)
            nc.vector.tensor_scalar_mul(fs[:], fs[:], 1.0 / QP)
            fs_col = scp.tile([128, 1], f32, tag="sc")
            nc.gpsimd.partition_broadcast(fs_col[:], fs[:])
            # r row for the quantization broadcast
            r_row = rowp.tile([1, t_len], f32, tag="trow")
            nc.sync.dma_start(r_row[0:1, :], rhalf[:])

            # ---------------- phase 2 pools (open after x_sb freed) ---------
            wstgb = stk.enter_context(tc.tile_pool(name="wstgb", bufs=2))
            wqp = stk.enter_context(tc.tile_pool(name="wqp", bufs=1))
            qp = stk.enter_context(tc.tile_pool(name="qp", bufs=1))
            xsp = stk.enter_context(tc.tile_pool(name="xsp", bufs=3))
            outp = stk.enter_context(tc.tile_pool(name="outp", bufs=4))

            # ---------------- weight pass B: ternary quant ------------------
            wq_sb = wqp.tile([128, WQ_F], bf16)
            for e in range(NW):
                w8 = wstgb.tile([128, WCH], f32, tag="wstgb")
                nc.sync.dma_start(w8[:], wt_t[:, ts(e, WCH)])
                nc.vector.tensor_scalar(w8[:], w8[:], winv_col[:], 1.0,
                                        op0=Alu.mult, op1=Alu.min)
                nc.vector.tensor_scalar(w8[:], w8[:], -1.0, C_MAGIC,
                                        op0=Alu.max, op1=Alu.add)
                nc.vector.tensor_scalar(wq_sb[:, ts(e, WCH)], w8[:],
                                        C_MAGIC, None, op0=Alu.subtract)
            wqv = wq_sb[:].rearrange("p (cb k ci o) -> p cb k ci o",
                                     cb=CB_BLOCKS, k=K, ci=CI_CHUNKS)

            # ---------------- phase 2: quantize activations -----------------
            q_sb = qp.tile([128, CI_CHUNKS, t_len], bf16)
            for j in range(NT):
                for ci in range(CI_CHUNKS):
                    xs = xsp.tile([128, TT], f32, tag="xs")
                    nc.sync.dma_start(xs[:], xv[:, ci, ts(j, TT)])
                    mb2 = ps_mb.tile([128, TT], f32, tag="mb")
                    nc.tensor.matmul(mb2[:], g2q_row[0:1, ts(ci, 128)],
                                     r_row[0:1, ts(j, TT)],
                                     start=True, stop=True)
                    u2 = scr.tile([128, TT], f32, tag="scr")
                    nc.vector.tensor_tensor(u2[:], xs[:], mb2[:], op=Alu.mult)
                    nc.vector.tensor_scalar(q_sb[:, ci, ts(j, TT)], u2[:],
                                            C_MAGIC, C_MAGIC,
                                            op0=Alu.add, op1=Alu.subtract)

            # ---------------- conv: 28 shifted matmuls per tile -------------
            # Tap order puts k=3 (always full width) first so the start=True
            # matmul covers the whole PSUM tile.
            tap_order = [3, 0, 1, 2, 4, 5, 6]
            for cb in range(CB_BLOCKS):
                for j in range(NT):
                    cps = ps_conv.tile([128, TT], f32, tag="conv")
                    n_mm = 0
                    for k in tap_order:
                        lo_data = j * TT + k - PAD
                        out_lo = max(0, -lo_data)
                        out_hi = TT - max(0, lo_data + TT - t_len)
                        for ci in range(CI_CHUNKS):
                            nc.tensor.matmul(
                                cps[:, out_lo:out_hi],
                                wqv[:, cb, k, ci, :],
                                q_sb[:, ci,
                                     lo_data + out_lo:lo_data + out_hi],
                                start=(n_mm == 0),
                                stop=(n_mm == K * CI_CHUNKS - 1))
                            n_mm += 1
                    osb = outp.tile([128, TT], f32)
                    nc.scalar.activation(osb[:], cps[:], Act.Copy,
                                         scale=fs_col[:])
                    nc.sync.dma_start(out_t[ts(cb, 128), ts(j, TT)], osb[:])

    nc.compile()
    return nc


def _prep_weight(weight: np.ndarray) -> np.ndarray:
    # WT[p, cb, k, ci, o'] = weight[cb*128+o', ci*128+p, k], flattened to
    # (128, 14336) so lhsT tiles are contiguous slices.
    w = np.ascontiguousarray(weight.astype(np.float32, copy=False))
    w5 = w.reshape(CB_BLOCKS, 128, CI_CHUNKS, 128, K)  # [cb, o', ci, p, k]
    wt = w5.transpose(3, 0, 4, 2, 1)  # [p, cb, k, ci, o']
    return np.ascontiguousarray(wt.reshape(128, -1))


def kernel(x: np.ndarray, weight: np.ndarray, gamma: np.ndarray) -> np.ndarray:
    from concourse.bass_utils import run_bass_kernel_spmd

    key = ("full", N_CORES, T)
    if key not in _CACHE:
        _CACHE[key] = _build(N_CORES, T)
    nc = _CACHE[key]

    wt = _prep_weight(weight)
    g = np.ascontiguousarray(gamma.astype(np.float32, copy=False))
    in_maps = [
        {"x": np.ascontiguousarray(x[b].astype(np.float32, copy=False)),
         "wt": wt, "g": g}
        for b in range(N_CORES)
    ]
    res = run_bass_kernel_spmd(nc, in_maps, list(range(N_CORES)))
    out = np.stack([res.results[b]["out"] for b in range(N_CORES)], axis=0)
    return out


# revision 21
# speedup vs baseline: 129.1670x; 129.1670x over previous
"""BitConv1d Trainium2 kernel.

Computes, for x:(8,512,8192) f32, weight:(512,512,7) f32, gamma:(512,) f32:
  rms  = sqrt(mean(x^2, channel) + 1e-6)          (per b,t)
  xn   = x / rms * gamma
  s    = max(|xn|) over the FULL batch  (clamped to >= 1e-5)
  q    = round(clip(xn/s*127, -128, 127))         (8-bit act quant, STE forward)
  ws   = max(mean(|w|), 1e-5); wq = round(clip(w/ws, -1, 1))  (ternary weights)
  out  = conv1d(q * s/127, wq, pad 3) * ws

Strategy: data-parallel over batch across 8 NeuronCores (1 batch element per
core), weights replicated. The activation-quant global max uses an on-device
AllReduce(max) of max(xn^2). The conv runs as 28 shifted bf16 matmuls per
output tile with exact integer arithmetic (q in [-127,127] and wq in {-1,0,1}
are exact in bf16; f32 PSUM accumulation of integers < 2^24 is exact), so the
conv result equals the integer conv scaled by s*ws/127. Rounding uses the
(x + 1.5*2^23) - 1.5*2^23 trick (round-half-even, matching jnp.round).
"""

import sys

sys.path.insert(0, "/opt/trn_rl_repo")

import numpy as np

N_CORES = 8
B, C, T = 8, 512, 8192
CO, K = 512, 7
CI_CHUNKS = 4  # 512 in-channels / 128 partitions
CB_BLOCKS = 4  # 512 out-channels / 128 partitions
TT = 512  # time-tile (columns per matmul)
PAD = 3  # conv padding

EPS_NORM = 1e-6
EPS_SCALE = 1e-5
QP = 127.0
C_MAGIC = 12582912.0  # 1.5 * 2^23 : (x + C) - C == round-half-even(x)
W_COUNT = CO * C * K

_CACHE = {}


def _build(n_cores: int, t_len: int):
    import contextlib
    import os
    skip_conv = os.environ.get("BITCONV_SKIP_CONV") == "1"
    skip_phase1 = os.environ.get("BITCONV_SKIP_PHASE1") == "1"
    skip_quant = os.environ.get("BITCONV_SKIP_QUANT") == "1"
    skip_w = os.environ.get("BITCONV_SKIP_W") == "1"
    skip_1a = os.environ.get("BITCONV_SKIP_1A") == "1"

    import concourse.bacc as bacc
    import concourse.bass as bass
    import concourse.tile as tile
    from concourse import bass_isa, mybir

    f32 = mybir.dt.float32
    bf16 = mybir.dt.bfloat16
    Alu = mybir.AluOpType
    Act = mybir.ActivationFunctionType
    ts = bass.ts

    NT = t_len // TT  # time tiles
    WQ_F = CB_BLOCKS * K * CI_CHUNKS * 128  # 14336
    NW = 16  # weight streaming chunks
    WCH = WQ_F // NW  # 896 columns per chunk

    nc = bacc.Bacc("TRN2", target_bir_lowering=False, debug=False,
                   num_devices=n_cores)

    x_t = nc.dram_tensor("x", [C, t_len], f32, kind="ExternalInput")
    wt_t = nc.dram_tensor("wt", [128, WQ_F], f32, kind="ExternalInput")
    g_t = nc.dram_tensor("g", [C], f32, kind="ExternalInput")
    out_t = nc.dram_tensor("out", [CO, t_len], f32, kind="ExternalOutput")

    xv = x_t[:].rearrange("(c p) t -> p c t", p=128)  # chunk-major channels

    with tile.TileContext(nc) as tc:
        with contextlib.ExitStack() as stk:
            singles = stk.enter_context(tc.tile_pool(name="singles", bufs=1))
            scr = stk.enter_context(tc.tile_pool(name="scr", bufs=3))
            bncp = stk.enter_context(tc.tile_pool(name="bncp", bufs=2))
            rmathp = stk.enter_context(tc.tile_pool(name="rmathp", bufs=5))
            scp = stk.enter_context(tc.tile_pool(name="scp", bufs=14))
            amaxp = stk.enter_context(tc.tile_pool(name="amaxp", bufs=2))
            rowp = stk.enter_context(tc.tile_pool(name="rowp", bufs=1))
            wstga = stk.enter_context(tc.tile_pool(name="wstga", bufs=2))
            dramp = stk.enter_context(
                tc.tile_pool(name="dram", bufs=1, space="DRAM"))
            ps_small = stk.enter_context(
                tc.tile_pool(name="ps_small", bufs=2, space="PSUM"))
            ps_mb = stk.enter_context(
                tc.tile_pool(name="ps_mb", bufs=2, space="PSUM"))
            ps_conv = stk.enter_context(
                tc.tile_pool(name="ps_conv", bufs=4, space="PSUM"))

            ones_col = singles.tile([128, 1], f32)
            nc.vector.memset(ones_col[:], 1.0)
            eps_col = singles.tile([128, 1], f32)
            nc.vector.memset(eps_col[:], EPS_NORM)
            zero_col = singles.tile([128, 1], f32)
            nc.vector.memset(zero_col[:], 0.0)
            g_row = singles.tile([1, C], f32)
            nc.sync.dma_start(g_row[:], g_t[:].rearrange("(a d) -> a d", a=1))

            cc_in = dramp.tile([128], f32)
            cc_out = dramp.tile([128], f32)

            FW = t_len // 128  # per-t arrays reshaped to (128, FW)
            PPT = TT // FW  # partitions covered by one t-tile

            # ---------------- phase 1a: sum of squares per t ---------------
            # x stays resident in SBUF through phase 1b.
            with tc.tile_pool(name="xres", bufs=1) as xres:
                x_sb = xres.tile([128, CI_CHUNKS, t_len], f32)
                rcol = singles.tile([128, FW], f32)  # channel-summed x^2
                for j in range(NT):
                    nc.sync.dma_start(x_sb[:, :, ts(j, TT)], xv[:, :, ts(j, TT)])
                    if skip_1a:
                        continue
                    ssq = ps_small.tile([1, TT], f32, tag="ssq")
                    for ci in range(CI_CHUNKS):
                        x2 = scr.tile([128, TT], f32, tag="scr")
                        nc.scalar.activation(x2[:], x_sb[:, ci, ts(j, TT)],
                                             Act.Square)
                        nc.tensor.matmul(ssq[:], ones_col[:], x2[:],
                                         start=(ci == 0),
                                         stop=(ci == CI_CHUNKS - 1))
                    sbounce = bncp.tile([1, TT], f32, tag="sbounce")
                    nc.vector.tensor_copy(sbounce[:], ssq[:])
                    nc.sync.dma_start(rcol[PPT * j:PPT * (j + 1), :],
                                      sbounce[:])

                # ---- r = 1/(2*rms), one Newton refinement of sqrt ----
                mcol = rmathp.tile([128, FW], f32, tag="rmath")
                s0 = rmathp.tile([128, FW], f32, tag="rmath")
                tdiv = rmathp.tile([128, FW], f32, tag="rmath")
                rhalf = rmathp.tile([128, FW], f32, tag="rmath")
                nc.vector.tensor_scalar(mcol[:], rcol[:], 1.0 / C, EPS_NORM,
                                        op0=Alu.mult, op1=Alu.add)
                nc.scalar.activation(s0[:], rcol[:], Act.Sqrt,
                                     bias=eps_col[:], scale=1.0 / C)
                nc.vector.reciprocal(tdiv[:], s0[:])
                nc.vector.tensor_tensor(tdiv[:], mcol[:], tdiv[:], op=Alu.mult)
                nc.vector.tensor_tensor(tdiv[:], tdiv[:], s0[:], op=Alu.add)
                nc.vector.reciprocal(rhalf[:], tdiv[:])  # 1/(2*rms)

                g2_row = singles.tile([1, C], f32)
                nc.vector.tensor_scalar_mul(g2_row[:], g_row[:], 2.0)
                r_row = rowp.tile([1, t_len], f32, tag="trow")
                nc.sync.dma_start(r_row[0:1, :], rhalf[:])

                # ---- weight pass A: sum(|w|) for mean (overlaps 1b) ----
                wsacc = None
                if skip_w:
                    wsacc = scp.tile([128, 1], f32, tag="sc")
                    nc.vector.memset(wsacc[:], 1.0)
                for e in range(0 if skip_w else NW):
                    wt_e = wstga.tile([128, WCH], f32, tag="wstga")
                    nc.scalar.dma_start(wt_e[:], wt_t[:, ts(e, WCH)])
                    wsq = scp.tile([128, 1], f32, tag="sc")
                    nc.scalar.activation(wt_e[:], wt_e[:], Act.Abs,
                                         accum_out=wsq[:])
                    if wsacc is None:
                        wsacc = wsq
                    else:
                        nxt = scp.tile([128, 1], f32, tag="sc")
                        nc.vector.tensor_tensor(nxt[:], wsacc[:], wsq[:],
                                                op=Alu.add)
                        wsacc = nxt
                wsum_ps = ps_small.tile([1, 1], f32, tag="ssq")
                nc.tensor.matmul(wsum_ps[:], wsacc[:], ones_col[:, 0:1],
                                 start=True, stop=True)
                wscale = scp.tile([1, 1], f32, tag="sc")
                nc.scalar.copy(wscale[:], wsum_ps[:])
                nc.vector.tensor_scalar(wscale[:], wscale[:], 1.0 / W_COUNT,
                                        EPS_SCALE, op0=Alu.mult, op1=Alu.max)
                winv = scp.tile([1, 1], f32, tag="sc")
                nc.vector.reciprocal(winv[:], wscale[:])
                winv_col = scp.tile([128, 1], f32, tag="sc")
                nc.gpsimd.partition_broadcast(winv_col[:], winv[:])

                # ---------------- phase 1b: local max of |xn| --------------
                coll = singles.tile([128, NT * CI_CHUNKS], f32)
                nc.vector.memset(coll[:], 0.0)
                for j in range(0 if skip_phase1 else NT):
                    for ci in range(CI_CHUNKS):
                        mb = ps_mb.tile([128, TT], f32, tag="mb")
                        nc.tensor.matmul(mb[:], g2_row[0:1, ts(ci, 128)],
                                         r_row[0:1, ts(j, TT)],
                                         start=True, stop=True)
                        u = scr.tile([128, TT], f32, tag="scrb")
                        idx = j * CI_CHUNKS + ci
                        nc.vector.tensor_tensor(u[:], x_sb[:, ci, ts(j, TT)],
                                                mb[:], op=Alu.mult)
                        nc.vector.tensor_reduce(
                            coll[:, idx:idx + 1], u[:],
                            axis=mybir.AxisListType.X, op=Alu.max,
                            apply_absolute_value=True)
                prev = amaxp.tile([128, 1], f32, tag="amax")
                nc.vector.tensor_reduce(prev[:], coll[:],
                                        axis=mybir.AxisListType.X, op=Alu.max)

            # x_sb freed here.
            amax_all = scp.tile([128, 1], f32, tag="sc")
            nc.gpsimd.partition_all_reduce(amax_all[:], prev[:], channels=128,
                                           reduce_op=bass_isa.ReduceOp.max)
            nc.sync.dma_start(cc_in[:], amax_all[:])
            if n_cores > 1:
                nc.gpsimd.collective_compute(
                    "AllReduce", Alu.max,
                    replica_groups=[list(range(n_cores))],
                    ins=[cc_in[:].opt()], outs=[cc_out[:].opt()])
            else:
                nc.sync.dma_start(cc_out[:], cc_in[:])

            v_raw = scp.tile([1, 1], f32, tag="sc")
            nc.sync.dma_start(v_raw[0:1, 0:1],
                              cc_out[0:1].rearrange("(a d) -> a d", a=1))
            qscale = scp.tile([1, 1], f32, tag="sc")
            nc.vector.tensor_scalar_max(qscale[:], v_raw[:], EPS_SCALE)
            qinv = scp.tile([1, 1], f32, tag="sc")
            nc.vector.reciprocal(qinv[:], qscale[:])
            q254 = scp.tile([1, 1], f32, tag="sc")
            nc.vector.tensor_scalar_mul(q254[:], qinv[:], 2.0 * QP)
            g2q_row = singles.tile([1, C], f32)
            nc.vector.tensor_scalar_mul(g2q_row[:], g_row[:], q254[:])
            # final output scale = wscale * qscale / 127
            fs = scp.tile([1, 1], f32, tag="sc")
            nc.vector.tensor_tensor(fs[:], wscale[:], qscale[:], op=Alu.mult)
            nc.vector.tensor_scalar_mul(fs[:], fs[:], 1.0 / QP)
            fs_col = scp.tile([128, 1], f32, tag="sc")
            nc.gpsimd.partition_broadcast(fs_col[:], fs[:])
            # ---------------- phase 2 pools (open after x_sb freed) ---------
            wstgb = stk.enter_context(tc.tile_pool(name="wstgb", bufs=2))
            wqp = stk.enter_context(tc.tile_pool(name="wqp", bufs=1))
            qp = stk.enter_context(tc.tile_pool(name="qp", bufs=1))
            xsp = stk.enter_context(tc.tile_pool(name="xsp", bufs=3))
            outp = stk.enter_context(tc.tile_pool(name="outp", bufs=4))

            # ---------------- weight pass B: ternary quant ------------------
            wq_sb = wqp.tile([128, WQ_F], bf16)
            for e in range(0 if skip_w else NW):
                w8 = wstgb.tile([128, WCH], f32, tag="wstgb")
                nc.scalar.dma_start(w8[:], wt_t[:, ts(e, WCH)])
                nc.vector.tensor_scalar(w8[:], w8[:], winv_col[:], 1.0,
                                        op0=Alu.mult, op1=Alu.min)
                nc.vector.tensor_scalar(w8[:], w8[:], -1.0, C_MAGIC,
                                        op0=Alu.max, op1=Alu.add)
                nc.vector.tensor_scalar(wq_sb[:, ts(e, WCH)], w8[:],
                                        C_MAGIC, None, op0=Alu.subtract)
            wqv = wq_sb[:].rearrange("p (cb k ci o) -> p cb k ci o",
                                     cb=CB_BLOCKS, k=K, ci=CI_CHUNKS)

            # ---------------- phase 2: quantize activations -----------------
            q_sb = qp.tile([128, CI_CHUNKS, t_len], bf16)
            for j in range(0 if skip_quant else NT):
                for ci in range(CI_CHUNKS):
                    xs = xsp.tile([128, TT], f32, tag="xs")
                    nc.sync.dma_start(xs[:], xv[:, ci, ts(j, TT)])
                    mb2 = ps_mb.tile([128, TT], f32, tag="mb")
                    nc.tensor.matmul(mb2[:], g2q_row[0:1, ts(ci, 128)],
                                     r_row[0:1, ts(j, TT)],
                                     start=True, stop=True)
                    u2 = scr.tile([128, TT], f32, tag="scr")
                    nc.vector.tensor_tensor(u2[:], xs[:], mb2[:], op=Alu.mult)
                    last_quant = nc.vector.tensor_scalar(
                        q_sb[:, ci, ts(j, TT)], u2[:], C_MAGIC, C_MAGIC,
                        op0=Alu.add, op1=Alu.subtract)

            # ---------------- conv: 28 shifted matmuls per tile -------------
            # Tap order puts k=3 (always full width) first so the start=True
            # matmul covers the whole PSUM tile.
            tap_order = [3, 0, 1, 2, 4, 5, 6]
            from concourse.bass import _add_dep_helper
            for cb in range(CB_BLOCKS if not skip_conv else 0):
                for j in range(NT):
                    cps = ps_conv.tile([128, TT], f32, tag="conv")
                    n_mm = 0
                    for k in tap_order:
                        lo_data = j * TT + k - PAD
                        out_lo = max(0, -lo_data)
                        out_hi = TT - max(0, lo_data + TT - t_len)
                        for ci in range(CI_CHUNKS):
                            mm = nc.tensor.matmul(
                                cps[:, out_lo:out_hi],
                                wqv[:, cb, k, ci, :],
                                q_sb[:, ci,
                                     lo_data + out_lo:lo_data + out_hi],
                                start=(n_mm == 0),
                                stop=(n_mm == K * CI_CHUNKS - 1))
                            if n_mm == 0 and not skip_quant:
                                # keep the conv MM stream dense: start only
                                # after quantization fully completes
                                _add_dep_helper(mm.ins, last_quant.ins, True,
                                                "conv after quant")
                            n_mm += 1
                    osb = outp.tile([128, TT], f32)
                    nc.scalar.activation(osb[:], cps[:], Act.Copy,
                                         scale=fs_col[:])
                    nc.scalar.dma_start(out_t[ts(cb, 128), ts(j, TT)], osb[:])

    nc.compile()
    return nc


def _prep_weight(weight: np.ndarray) -> np.ndarray:
    # WT[p, cb, k, ci, o'] = weight[cb*128+o', ci*128+p, k], flattened to
    # (128, 14336) so lhsT tiles are contiguous slices.
    w = np.ascontiguousarray(weight.astype(np.float32, copy=False))
    w5 = w.reshape(CB_BLOCKS, 128, CI_CHUNKS, 128, K)  # [cb, o', ci, p, k]
    wt = w5.transpose(3, 0, 4, 2, 1)  # [p, cb, k, ci, o']
    return np.ascontiguousarray(wt.reshape(128, -1))


def kernel(x: np.ndarray, weight: np.ndarray, gamma: np.ndarray) -> np.ndarray:
    from concourse.bass_utils import run_bass_kernel_spmd

    key = ("full", N_CORES, T)
    if key not in _CACHE:
        _CACHE[key] = _build(N_CORES, T)
    nc = _CACHE[key]

    wt = _prep_weight(weight)
    g = np.ascontiguousarray(gamma.astype(np.float32, copy=False))
    in_maps = [
        {"x": np.ascontiguousarray(x[b].astype(np.float32, copy=False)),
         "wt": wt, "g": g}
        for b in range(N_CORES)
    ]
    res = run_bass_kernel_spmd(nc, in_maps, list(range(N_CORES)))
    out = np.stack([res.results[b]["out"] for b in range(N_CORES)], axis=0)
    return out
